# revision 9
# baseline (speedup 1.0000x reference)
"""Self-contained TRN2 Bass kernel for the GCN message-passing problem.

8-core SPMD, v6 (SBUF-table gather architecture):
- Nodes sharded by dst across cores (NS = N/C per core). GCN norm
  (dinv_src*dinv_dst) folded into the one-hot selection matrices.
- Per-layer node-feature table h (bf16, [Npad, 64] DRAM, core/chunk-major
  "AG layout") viewed as 50176 tokens of 128 bf16 (2 nodes per token).
  Each layer the full table is loaded into SBUF (98KB/partition, blocked:
  token q -> partition loc//196, stripe loc%196, per 25088-token half).
- Edges grouped by (dst block, half, parity) into 128-edge tiles; tiles
  gathered from the SBUF table with custom transpose-mode dma_gather
  (<=512 rows/call, int16 indices, round-robin over 4 SWDGE queues:
  ~0.9ns/row). Gathered columns are feature-major; per tile: PE transpose
  (bf16, into grouped psum), ACT copy psum->SBUF, DVE one-hot
  (is_equal x norm), PE matmul psum[64f,128d] += mt[128e,64].T @ oh.
- Per-block transform: psum_agg -> fp16 tile; h' = relu(agg @ W + b) via
  two fp16 matmuls (ones-row trick for bias) -> node-major psum -> ACT
  relu -> bf16 -> bounce. Tables for layers 1,2 distributed via chunked
  AllGathers interleaved with block processing. Layer-0 table computed
  fully on every core (replicated embed, fp16, no collective).
- Pooling: one-hot over G graphs into psum [64, G], AllReduce, small fp32
  MLP on every core; core 0's output used.
"""
from dataclasses import dataclass

import numpy as np
import jax
import ml_dtypes
from jax.sharding import Mesh, PartitionSpec
from jax.experimental.shard_map import shard_map

from concourse import bass2jax
from concourse.bass2jax import _bass_exec_p, install_neuronx_cc_hook
from concourse import library_config

import concourse.bass as bass
import concourse.bacc as bacc
import concourse.mybir as mybir
import concourse.tile as tile

F32 = mybir.dt.float32
BF16 = mybir.dt.bfloat16
FP16 = mybir.dt.float16
I16 = mybir.dt.int16

BF = ml_dtypes.bfloat16
F16 = np.float16

HTOK = 25088          # tokens per half (196 stripes x 128 partitions)
NSTRH = 196           # stripes per half per partition
MAXCALL = 4           # tiles per dma_gather call (<=512 idx)


@dataclass
class Meta:
    N: int
    F: int
    H: int
    G: int
    L: int
    C: int
    NS: int
    NB: int
    NBF: int
    T_tot: int
    NCHK: int
    BPC: int
    block_calls: tuple   # per block: tuple of (nt, half)
    block_par: tuple     # per block: tuple of parity per tile
    nq: int = 4


def preprocess(x, edge_index, batch, W_emb, b_emb, conv_W, conv_b,
               W1, b1, W2, b2, W3, b3, n_cores=8, G=None, NCHK=5):
    x = np.asarray(x, np.float32)
    ei = np.asarray(edge_index, np.int64)
    batch = np.asarray(batch, np.int64)
    N, F = x.shape
    H = int(np.asarray(W_emb).shape[1])
    L = int(np.asarray(conv_W).shape[0])
    C = n_cores
    NS = N // C
    NB = (NS + 127) // 128
    NBF = (N + 127) // 128

    if G is None:
        G = int(batch.max()) + 1 if batch.size else 1

    loop = np.arange(N, dtype=np.int64)
    deg = (np.bincount(np.concatenate([ei[1], loop]), minlength=N)
           .astype(np.float64))
    dinv = (1.0 / np.sqrt(np.maximum(deg, 1.0))).astype(np.float32)
    src = np.concatenate([ei[0], loop])
    dst = np.concatenate([ei[1], loop])

    # ---- table ("AG") layout: chunk-major, core-major within chunk ----
    BPC = (NB + NCHK - 1) // NCHK
    CSZ = BPC * 128
    rk = np.array([min(NS, (k + 1) * CSZ) - min(NS, k * CSZ)
                   for k in range(NCHK)], np.int64)
    base = np.zeros(NCHK, np.int64)
    base[1:] = np.cumsum(C * rk)[:-1]

    def tablepos(v):
        c2 = v // NS
        r = v % NS
        k = np.minimum(r // CSZ, NCHK - 1)
        return base[k] + c2 * rk[k] + (r - k * CSZ)

    tp_all = tablepos(np.arange(N, dtype=np.int64))
    perm = np.zeros(N, np.int64)
    perm[tp_all] = np.arange(N)          # perm[tablepos] = original node

    # per-edge gather coordinates
    tp_src = tablepos(src)
    tok = tp_src // 2
    par = (tp_src % 2).astype(np.int64)
    half = tok // HTOK
    loc = tok % HTOK
    i16 = ((loc % NSTRH) * 128 + loc // NSTRH).astype(np.int16)
    norm = (dinv[src] * dinv[dst]).astype(np.float32)

    core = dst // NS
    block = (dst % NS) // 128
    dloc = (dst % NS - block * 128).astype(np.float32)

    # sort edges by (core, block, half, parity)
    order = np.lexsort((par, half, block, core))
    core_s = core[order]
    block_s = block[order]
    half_s = half[order]
    par_s = par[order]
    i16_s = i16[order]
    dloc_s = dloc[order]
    norm_s = norm[order]

    in_maps = []
    block_calls_all = None
    block_par_all = None
    T_tot_all = None
    idx_cols_all = None

    per_core = []
    for c in range(C):
        msk = core_s == c
        per_core.append((block_s[msk], half_s[msk], par_s[msk],
                         i16_s[msk], dloc_s[msk], norm_s[msk]))

    # build per-core tile lists; tbb/calls must be IDENTICAL across cores
    # (same program), so compute per-(block, half, parity) padded tile
    # counts as the max over cores.
    nsub = np.zeros((C, NB, 2, 2), np.int64)
    for c in range(C):
        bs, hs, ps, _, _, _ = per_core[c]
        np.add.at(nsub, (c, bs, hs, ps), 1)
    tsub = (nsub.max(axis=0) + 127) // 128          # [NB, 2, 2] tiles
    tsub = np.maximum(tsub, 0)

    block_calls = []
    block_par = []
    for b in range(NB):
        calls = []
        pars = []
        for h in range(2):
            tl = []                                  # parities per tile
            for p in range(2):
                tl += [p] * int(tsub[b, h, p])
            for o in range(0, len(tl), MAXCALL):
                nt = min(MAXCALL, len(tl) - o)
                calls.append((nt, h))
            pars += tl
        block_calls.append(tuple(calls))
        block_par.append(tuple(pars))
    T_tot = int(sum(len(p) for p in block_par))

    iota128 = np.tile(np.arange(128, dtype=np.float32), (128, 1)).astype(BF)
    iotag = np.tile(np.arange(G, dtype=np.float32), (128, 1)).astype(BF)
    ident = np.eye(128, dtype=np.float32).astype(BF)
    onesrow = np.ones((1, 128), F16)

    # replicated-embed inputs, permuted to table order, fp16
    xT1 = np.ones((F + 1, N), np.float32)
    xT1[:F] = x.T
    xT1 = np.ascontiguousarray(xT1[:, perm]).astype(F16)
    wemb1 = np.concatenate(
        [np.asarray(W_emb, np.float32),
         np.asarray(b_emb, np.float32).reshape(1, H)], axis=0).astype(F16)

    conv_W = np.asarray(conv_W, np.float32)
    conv_b = np.asarray(conv_b, np.float32)

    cntg = np.bincount(batch, minlength=G).astype(np.float32)
    invc = np.tile((1.0 / np.maximum(cntg, 1.0))[None, :],
                   (64, 1)).astype(np.float32)

    meta = Meta(N=N, F=F, H=H, G=G, L=L, C=C, NS=NS, NB=NB, NBF=NBF,
                T_tot=T_tot, NCHK=NCHK, BPC=BPC,
                block_calls=tuple(block_calls), block_par=tuple(block_par))

    for c in range(C):
        bs, hs, ps, i16c, dlc, nmc = per_core[c]
        # bucket edges
        buckets = {}
        for b in range(NB):
            for h in range(2):
                for p in range(2):
                    buckets[(b, h, p)] = []
        bsel = {}
        for key in buckets:
            b, h, p = key
            m = (bs == b) & (hs == h) & (ps == p)
            bsel[key] = (i16c[m], dlc[m], nmc[m])

        idxw_parts = []
        dstloc_arr = np.full((128, T_tot), -1.0, np.float32)
        norm_arr = np.zeros((128, T_tot), np.float32)
        t_glob = 0
        for b in range(NB):
            for h in range(2):
                # concatenated (parity 0 tiles, parity 1 tiles) edge stream
                tiles_idx = []
                for p in range(2):
                    ii, dd, nn = bsel[(b, h, p)]
                    ntile = int(tsub[b, h, p])
                    n = len(ii)
                    padn = ntile * 128
                    ip = np.zeros(padn, np.int16)
                    ip[:n] = ii
                    if n < padn:
                        ip[n:] = ii[-1] if n else 0
                    dp = np.full(padn, -1.0, np.float32)
                    dp[:n] = dd
                    np_ = np.zeros(padn, np.float32)
                    np_[:n] = nn
                    for t in range(ntile):
                        sl = slice(t * 128, (t + 1) * 128)
                        dstloc_arr[:, t_glob] = dp[sl]
                        norm_arr[:, t_glob] = np_[sl]
                        tiles_idx.append(ip[sl])
                        t_glob += 1
                # gather calls over these tiles (<=MAXCALL each)
                for o in range(0, len(tiles_idx), MAXCALL):
                    grp = tiles_idx[o:o + MAXCALL]
                    flat = np.concatenate(grp)
                    w = flat.reshape(len(flat) // 16, 16).T
                    idxw_parts.append(np.tile(w, (8, 1)))
        assert t_glob == T_tot
        idxw = np.concatenate(idxw_parts, axis=1)
        assert idxw.shape == (128, T_tot * 8)

        poolid = np.full((128, NB), -1.0, np.float32)
        for b in range(NB):
            w = min(128, NS - b * 128)
            poolid[:w, b] = batch[c * NS + b * 128: c * NS + b * 128 + w]

        m = {
            "xt1": xT1,
            "idxw": np.ascontiguousarray(idxw),
            "dstloc": dstloc_arr,
            "norme": norm_arr,
            "poolid": poolid,
            "iota128": iota128,
            "iotag": iotag,
            "ident": ident,
            "onesrow": onesrow,
            "wemb1": wemb1,
            "invc": invc,
            "w1": np.asarray(W1, np.float32),
            "b1": np.asarray(b1, np.float32).reshape(-1, 1),
            "w2": np.asarray(W2, np.float32),
            "b2": np.asarray(b2, np.float32).reshape(-1, 1),
            "w3": np.asarray(W3, np.float32),
            "b3": np.asarray(b3, np.float32).reshape(1, 1),
        }
        for i in range(L):
            m[f"cw_{i}"] = conv_W[i].astype(F16)
            m[f"cb_{i}"] = conv_b[i].reshape(1, H).astype(F16)
        in_maps.append(m)
    return meta, in_maps


def build_nc(meta: Meta, repeats=1):
    N, F, H, G, L, C = meta.N, meta.F, meta.H, meta.G, meta.L, meta.C
    NS, NB = meta.NS, meta.NB
    T_tot = meta.T_tot
    NCHK, BPC, nq = meta.NCHK, meta.BPC, meta.nq
    NPAD = 2 * 2 * HTOK                  # padded table rows (100352)
    CSZ = BPC * 128

    nc = bacc.Bacc("TRN2", target_bir_lowering=False, debug=False,
                   num_devices=C, num_swdge_queues=nq,
                   dynamic_dma_scratch_size=16384)

    def EIN(name, shape, dt):
        return nc.dram_tensor(name, list(shape), dt, kind="ExternalInput")

    xt1 = EIN("xt1", [F + 1, N], FP16)
    idxw = EIN("idxw", [128, T_tot * 8], I16)
    dstloc = EIN("dstloc", [128, T_tot], F32)
    norme = EIN("norme", [128, T_tot], F32)
    poolid = EIN("poolid", [128, NB], F32)
    iota128 = EIN("iota128", [128, 128], BF16)
    iotag = EIN("iotag", [128, G], BF16)
    ident = EIN("ident", [128, 128], BF16)
    onesrow = EIN("onesrow", [1, 128], FP16)
    wemb1 = EIN("wemb1", [F + 1, H], FP16)
    invc = EIN("invc", [64, G], F32)
    w1 = EIN("w1", [H, H], F32)
    b1 = EIN("b1", [H, 1], F32)
    w2 = EIN("w2", [H, H // 2], F32)
    b2 = EIN("b2", [H // 2, 1], F32)
    w3 = EIN("w3", [H // 2, 1], F32)
    b3 = EIN("b3", [1, 1], F32)
    cw = [EIN(f"cw_{i}", [H, H], FP16) for i in range(L)]
    cb = [EIN(f"cb_{i}", [1, H], FP16) for i in range(L)]

    out_d = nc.dram_tensor("out", [1, G], F32, kind="ExternalOutput")

    table0 = nc.dram_tensor("table0", [NPAD, H], BF16)
    table1 = nc.dram_tensor("table1", [NPAD, H], BF16, addr_space="Shared")
    table2 = nc.dram_tensor("table2", [NPAD, H], BF16, addr_space="Shared")
    bounce = nc.dram_tensor("bounce", [NS, H], BF16)
    pool_in = nc.dram_tensor("pool_in", [H, G], F32)
    pool_out = nc.dram_tensor("pool_out", [H, G], F32, addr_space="Shared")

    groups = [list(range(C))]
    chunk_lim = [(min(NS, k * CSZ), min(NS, (k + 1) * CSZ))
                 for k in range(NCHK)]

    gq = [0]

    with tile.TileContext(nc) as tc:
        import contextlib
        ctx = contextlib.ExitStack()
        with ctx:
            P = ctx.enter_context
            persist = P(tc.tile_pool(name="persist", bufs=1))
            gpool = P(tc.tile_pool(name="gpool", bufs=6))
            mtpool = P(tc.tile_pool(name="mtpool", bufs=4))
            ohpool = P(tc.tile_pool(name="ohpool", bufs=12))
            hap = P(tc.tile_pool(name="hap", bufs=3))
            stpool = P(tc.tile_pool(name="stpool", bufs=4))
            pohpool = P(tc.tile_pool(name="pohpool", bufs=3))
            bp_ps = P(tc.tile_pool(name="bp_ps", bufs=3, space="PSUM"))
            tr_ps = P(tc.tile_pool(name="tr_ps", bufs=2, space="PSUM"))
            st_ps = P(tc.tile_pool(name="st_ps", bufs=2, space="PSUM"))

            nc.gpsimd.load_library(library_config.mlp)

            def load(name, ap, shape, dt):
                t = persist.tile(list(shape), dt, tag=name)
                nc.sync.dma_start(out=t[:], in_=ap[:])
                return t

            idx_sb = load("idx_sb", idxw, [128, T_tot * 8], I16)
            dstloc_sb = load("dstloc_sb", dstloc, [128, T_tot], F32)
            norme_sb = load("norme_sb", norme, [128, T_tot], F32)
            poolid_sb = load("poolid_sb", poolid, [128, NB], F32)
            iota_sb = load("iota_sb", iota128, [128, 128], BF16)
            iotag_sb = load("iotag_sb", iotag, [128, G], BF16)
            ident_sb = load("ident_sb", ident, [128, 128], BF16)
            ones_sb = load("ones_sb", onesrow, [1, 128], FP16)
            wemb1_sb = load("wemb1_sb", wemb1, [F + 1, H], FP16)
            invc_sb = load("invc_sb", invc, [64, G], F32)
            w1_sb = load("w1_sb", w1, [H, H], F32)
            b1_sb = load("b1_sb", b1, [H, 1], F32)
            w2_sb = load("w2_sb", w2, [H, H // 2], F32)
            b2_sb = load("b2_sb", b2, [H // 2, 1], F32)
            w3_sb = load("w3_sb", w3, [H // 2, 1], F32)
            b3_sb = load("b3_sb", b3, [1, 1], F32)
            cw_sb = [load(f"cw_{i}_sb", cw[i], [H, H], FP16)
                     for i in range(L)]
            cb_sb = [load(f"cb_{i}_sb", cb[i], [1, H], FP16)
                     for i in range(L)]

            tab_sb = persist.tile([128, 2 * HTOK], BF16, tag="tab_sb")
            h3n = persist.tile([128, NB * H], BF16, tag="h3n")

            def gather(g_ap, half, c0, nidx):
                nc.gpsimd.dma_gather(
                    g_ap, tab_sb[:, half * HTOK:(half + 1) * HTOK],
                    idx_sb[:, c0:c0 + nidx // 16], nidx, nidx, 128,
                    transpose=True,
                    sbuf_tokens_per_rank=128,
                    sbuf_free_dim_per_rank=256,
                    sbuf_free_dim_pad_per_rank=0,
                    sbuf_byte_offset=0,
                    queue_num=gq[0] % nq)
                gq[0] += 1

            for _rep in range(repeats):
                # ======== embed (replicated, table order, fp16) ========
                XCH = 512
                with tc.tile_pool(name="xpool", bufs=3) as xpool:
                    for c0 in range(0, N, XCH):
                        cwd = min(XCH, N - c0)
                        xt = xpool.tile([F + 1, XCH], FP16, tag="xt")
                        nc.sync.dma_start(out=xt[:, :cwd],
                                          in_=xt1[:, c0:c0 + cwd])
                        for o in range(0, cwd, 128):
                            w = min(128, cwd - o)
                            ps = st_ps.tile([128, H], F32, tag="stps")
                            nc.tensor.matmul(out=ps[:w, :],
                                             lhsT=xt[:, o:o + w],
                                             rhs=wemb1_sb[:],
                                             start=True, stop=True)
                            st = stpool.tile([128, H], BF16, tag="st")
                            nc.scalar.activation(
                                out=st[:w, :], in_=ps[:w, :],
                                func=mybir.ActivationFunctionType.Relu)
                            nc.sync.dma_start(
                                out=table0[c0 + o:c0 + o + w, :],
                                in_=st[:w, :])

                # ======== conv layers ========
                tables = [table0, table1, table2]
                for li in range(L):
                    t_in = tables[li]
                    # load table into SBUF (blocked token layout, per half)
                    for h in range(2):
                        nc.sync.dma_start(
                            out=tab_sb[:, h * HTOK:(h + 1) * HTOK],
                            in_=t_in[h * 2 * HTOK:(h + 1) * 2 * HTOK, :]
                            .rearrange("(p s) e -> p (s e)", p=128))
                    t_glob = 0
                    c_glob = 0     # idx column (wrapped, /16)
                    for b in range(NB):
                        w = min(128, NS - b * 128)
                        calls = meta.block_calls[b]
                        pars = meta.block_par[b]
                        tb = len(pars)
                        ps_agg = bp_ps.tile([64, 128], F32, tag="bps")
                        # gather all tiles of this block
                        gts = []    # (gtile, j_in_call) per tile
                        for (nt, h) in calls:
                            g = gpool.tile([128, MAXCALL * 128], BF16,
                                           tag="g")
                            gather(g[:, :nt * 128]
                                   .rearrange("p (c e) -> p c e", c=1),
                                   h, c_glob, nt * 128)
                            c_glob += nt * 8
                            for j in range(nt):
                                gts.append((g, j))
                        # process tiles in groups of 4 (one psum bank):
                        # transpose each tile via out = g.T @ I (fp32 psum)
                        ti = 0
                        for g0 in range(0, tb, 4):
                            grp = gts[g0:g0 + 4]
                            ng = len(grp)
                            trp = tr_ps.tile([128, 4 * 128], F32,
                                             tag="trp")
                            for jj, (g, j) in enumerate(grp):
                                nc.tensor.matmul(
                                    out=trp[:, jj * 128:(jj + 1) * 128],
                                    lhsT=g[:, j * 128:(j + 1) * 128],
                                    rhs=ident_sb[:],
                                    start=True, stop=True)
                            mt = mtpool.tile([128, 4 * 128], BF16, tag="mt")
                            nc.scalar.activation(
                                out=mt[:, :ng * 128], in_=trp[:, :ng * 128],
                                func=mybir.ActivationFunctionType.Copy)
                            for jj in range(ng):
                                pi = pars[g0 + jj]
                                tt = t_glob + g0 + jj
                                oh = ohpool.tile([128, 128], BF16, tag="oh")
                                nc.vector.tensor_scalar(
                                    out=oh[:], in0=iota_sb[:],
                                    scalar1=dstloc_sb[:, tt:tt + 1],
                                    scalar2=norme_sb[:, tt:tt + 1],
                                    op0=mybir.AluOpType.is_equal,
                                    op1=mybir.AluOpType.mult)
                                nc.tensor.matmul(
                                    out=ps_agg[:],
                                    lhsT=mt[:, jj * 128 + pi * 64:
                                            jj * 128 + pi * 64 + 64],
                                    rhs=oh[:],
                                    start=(ti == 0), stop=(ti == tb - 1))
                                ti += 1
                        t_glob += tb
                        # block transform: h' = relu(agg @ W + b)
                        ha = hap.tile([64, 128], FP16, tag="ha")
                        nc.vector.tensor_copy(out=ha[:, :], in_=ps_agg[:])
                        ps2 = st_ps.tile([128, H], F32, tag="stps")
                        nc.tensor.matmul(out=ps2[:w, :], lhsT=ha[:, :w],
                                         rhs=cw_sb[li][:],
                                         start=True, stop=False)
                        nc.tensor.matmul(out=ps2[:w, :],
                                         lhsT=ones_sb[:, :w],
                                         rhs=cb_sb[li][:],
                                         start=False, stop=True)
                        if li < L - 1:
                            st = stpool.tile([128, H], BF16, tag="st")
                            nc.scalar.activation(
                                out=st[:w, :], in_=ps2[:w, :],
                                func=mybir.ActivationFunctionType.Relu)
                            nc.sync.dma_start(
                                out=bounce[b * 128:b * 128 + w, :],
                                in_=st[:w, :])
                            for k in range(NCHK):
                                if b == min(NB, (k + 1) * BPC) - 1:
                                    r0, r1 = chunk_lim[k]
                                    if r1 > r0:
                                        t_out = tables[li + 1]
                                        nc.gpsimd.collective_compute(
                                            "AllGather",
                                            mybir.AluOpType.bypass,
                                            replica_groups=groups,
                                            ins=[bounce[r0:r1, :]],
                                            outs=[t_out[C * r0:C * r1, :]])
                        else:
                            nc.scalar.activation(
                                out=h3n[:w, b * H:(b + 1) * H],
                                in_=ps2[:w, :],
                                func=mybir.ActivationFunctionType.Relu)

                # ======== pooling ========
                with tc.tile_pool(name="pool_ps", bufs=1,
                                  space="PSUM") as pool_ps:
                    pps = pool_ps.tile([64, G], F32, tag="pps")
                    for b in range(NB):
                        w = min(128, NS - b * 128)
                        ohp = pohpool.tile([128, G], BF16, tag="ohp")
                        nc.vector.tensor_scalar(
                            out=ohp[:w, :], in0=iotag_sb[:w, :],
                            scalar1=poolid_sb[:w, b:b + 1], scalar2=None,
                            op0=mybir.AluOpType.is_equal)
                        nc.tensor.matmul(out=pps[:],
                                         lhsT=h3n[:w, b * H:(b + 1) * H],
                                         rhs=ohp[:w, :], start=(b == 0),
                                         stop=(b == NB - 1))
                    psum_sb = persist.tile([64, G], F32, tag="psum_sb")
                    nc.vector.tensor_copy(out=psum_sb[:], in_=pps[:])
                nc.sync.dma_start(out=pool_in[:], in_=psum_sb[:])
                nc.gpsimd.collective_compute(
                    "AllReduce", mybir.AluOpType.add, replica_groups=groups,
                    ins=[pool_in[:]], outs=[pool_out[:]])
                pooled = persist.tile([64, G], F32, tag="pooled")
                nc.sync.dma_start(out=pooled[:], in_=pool_out[:])
                nc.vector.tensor_tensor(out=pooled[:], in0=pooled[:],
                                        in1=invc_sb[:],
                                        op=mybir.AluOpType.mult)
                # ======== MLP ========
                with tc.tile_pool(name="mlp_ps", bufs=1,
                                  space="PSUM") as mlp_ps:
                    ps1 = mlp_ps.tile([64, G], F32, tag="mlpps")
                    nc.tensor.matmul(out=ps1[:, :G], lhsT=w1_sb[:],
                                     rhs=pooled[:], start=True, stop=True)
                    r1 = persist.tile([64, G], F32, tag="r1")
                    nc.scalar.activation(
                        out=r1[:], in_=ps1[:64, :G],
                        func=mybir.ActivationFunctionType.Relu,
                        bias=b1_sb[:, 0:1])
                    ps2m = mlp_ps.tile([64, G], F32, tag="mlpps")
                    nc.tensor.matmul(out=ps2m[:32, :G], lhsT=w2_sb[:],
                                     rhs=r1[:], start=True, stop=True)
                    r2 = persist.tile([32, G], F32, tag="r2")
                    nc.scalar.activation(
                        out=r2[:], in_=ps2m[:32, :G],
                        func=mybir.ActivationFunctionType.Relu,
                        bias=b2_sb[:, 0:1])
                    ps3 = mlp_ps.tile([64, G], F32, tag="mlpps")
                    nc.tensor.matmul(out=ps3[:1, :G], lhsT=w3_sb[:],
                                     rhs=r2[:], start=True, stop=True)
                    outs = persist.tile([1, G], F32, tag="outs")
                    nc.vector.tensor_scalar(out=outs[:], in0=ps3[:1, :G],
                                            scalar1=b3_sb[0:1, 0:1],
                                            scalar2=None,
                                            op0=mybir.AluOpType.add)
                nc.sync.dma_start(out=out_d[:], in_=outs[:])

    nc.compile()
    return nc


class SpmdRunner:
    def __init__(self, nc, n_cores):
        install_neuronx_cc_hook()
        self.nc = nc
        self.n_cores = n_cores
        partition_name = (nc.partition_id_tensor.name
                          if nc.partition_id_tensor else None)
        in_names, out_names, out_avals, zero_outs = [], [], [], []
        for alloc in nc.m.functions[0].allocations:
            if not isinstance(alloc, mybir.MemoryLocationSet):
                continue
            name = alloc.memorylocations[0].name
            if alloc.kind == "ExternalInput":
                if name != partition_name:
                    in_names.append(name)
            elif alloc.kind == "ExternalOutput":
                shape = tuple(alloc.tensor_shape)
                dt = mybir.dt.np(alloc.dtype)
                out_names.append(name)
                out_avals.append(jax.core.ShapedArray(shape, dt))
                zero_outs.append(np.zeros(shape, dt))
        self.in_names, self.out_names = in_names, out_names
        self.zero_outs = zero_outs
        bind_in_names = in_names + out_names
        if partition_name is not None:
            bind_in_names.append(partition_name)

        def _body(*args):
            operands = list(args)
            if partition_name is not None:
                operands.append(bass2jax.partition_id_tensor())
            outs = _bass_exec_p.bind(
                *operands,
                out_avals=tuple(out_avals),
                in_names=tuple(bind_in_names),
                out_names=tuple(out_names),
                lowering_input_output_aliases=(),
                sim_require_finite=False,
                sim_require_nnan=False,
                nc=nc,
            )
            return tuple(outs)

        devices = jax.devices()[:n_cores]
        self.mesh = Mesh(np.asarray(devices), ("core",))
        n_args = len(in_names) + len(zero_outs)
        in_specs = (PartitionSpec("core"),) * n_args
        out_specs = (PartitionSpec("core"),) * len(out_names)
        self.fn = jax.jit(
            shard_map(_body, mesh=self.mesh, in_specs=in_specs,
                      out_specs=out_specs, check_rep=False),
            keep_unused=True,
        )
        self._dev_in = None

    def set_inputs(self, in_maps):
        assert len(in_maps) == self.n_cores
        concat = [np.concatenate([np.asarray(in_maps[c][n])
                                  for c in range(self.n_cores)], axis=0)
                  for n in self.in_names]
        self._dev_in = [jax.device_put(a) for a in concat]
        self._dev_zeros = [
            jax.device_put(np.zeros((self.n_cores * z.shape[0], *z.shape[1:]),
                                    z.dtype)) for z in self.zero_outs]
        jax.block_until_ready(self._dev_in)

    def run(self):
        outs = self.fn(*self._dev_in, *self._dev_zeros)
        jax.block_until_ready(outs)
        return outs

    def results(self, outs):
        res = [dict() for _ in range(self.n_cores)]
        for i, name in enumerate(self.out_names):
            arr = np.asarray(outs[i])
            per = np.split(arr, self.n_cores, axis=0)
            for c in range(self.n_cores):
                res[c][name] = per[c]
        return res


_CACHE = {}


def _get_runner(meta, in_maps, repeats=1):
    key = (meta.N, meta.T_tot, meta.NCHK, hash(meta.block_calls),
           hash(meta.block_par), repeats)
    if key not in _CACHE:
        nc = build_nc(meta, repeats=repeats)
        _CACHE[key] = SpmdRunner(nc, meta.C)
    return _CACHE[key]


def kernel(x, edge_index, batch, W_emb, b_emb, conv_W, conv_b,
           W1, b1, W2, b2, W3, b3):
    """Full (unsharded) inputs -> full [G, 1] float32 output."""
    G = 256
    meta, in_maps = preprocess(
        x, edge_index, batch, W_emb, b_emb, conv_W, conv_b,
        W1, b1, W2, b2, W3, b3, n_cores=8, G=G)
    r = _get_runner(meta, in_maps)
    r.set_inputs(in_maps)
    res = r.results(r.run())
    return np.ascontiguousarray(res[0]["out"].reshape(G, 1).astype(np.float32))


# revision 11
# speedup vs baseline: 2.0987x; 2.0987x over previous
"""Self-contained TRN2 Bass kernel for the GCN message-passing problem.

8-core SPMD, v6 (SBUF-table gather architecture):
- Nodes sharded by dst across cores (NS = N/C per core). GCN norm
  (dinv_src*dinv_dst) folded into the one-hot selection matrices.
- Per-layer node-feature table h (bf16, [Npad, 64] DRAM, core/chunk-major
  "AG layout") viewed as 50176 tokens of 128 bf16 (2 nodes per token).
  Each layer the full table is loaded into SBUF (98KB/partition, blocked:
  token q -> partition loc//196, stripe loc%196, per 25088-token half).
- Edges grouped by (dst block, half, parity) into 128-edge tiles; tiles
  gathered from the SBUF table with custom transpose-mode dma_gather
  (<=512 rows/call, int16 indices, round-robin over 4 SWDGE queues:
  ~0.9ns/row). Gathered columns are feature-major; per tile: PE transpose
  (bf16, into grouped psum), ACT copy psum->SBUF, DVE one-hot
  (is_equal x norm), PE matmul psum[64f,128d] += mt[128e,64].T @ oh.
- Per-block transform: psum_agg -> fp16 tile; h' = relu(agg @ W + b) via
  two fp16 matmuls (ones-row trick for bias) -> node-major psum -> ACT
  relu -> bf16 -> bounce. Tables for layers 1,2 distributed via chunked
  AllGathers interleaved with block processing. Layer-0 table computed
  fully on every core (replicated embed, fp16, no collective).
- Pooling: one-hot over G graphs into psum [64, G], AllReduce, small fp32
  MLP on every core; core 0's output used.
"""
from dataclasses import dataclass

import numpy as np
import jax
import ml_dtypes
from jax.sharding import Mesh, PartitionSpec
from jax.experimental.shard_map import shard_map

from concourse import bass2jax
from concourse.bass2jax import _bass_exec_p, install_neuronx_cc_hook
from concourse import library_config

import concourse.bass as bass
import concourse.bacc as bacc
import concourse.mybir as mybir
import concourse.tile as tile

F32 = mybir.dt.float32
BF16 = mybir.dt.bfloat16
FP16 = mybir.dt.float16
I16 = mybir.dt.int16

BF = ml_dtypes.bfloat16
F16 = np.float16

HTOK = 25088          # tokens per half (196 stripes x 128 partitions)
NSTRH = 196           # stripes per half per partition
MAXCALL = 4           # tiles per dma_gather call (<=512 idx)


@dataclass
class Meta:
    N: int
    F: int
    H: int
    G: int
    L: int
    C: int
    NS: int
    NB: int
    NBF: int
    T_tot: int
    NCHK: int
    BPC: int
    block_calls: tuple   # per block: tuple of (nt, half)
    block_par: tuple     # per block: tuple of parity per tile
    nq: int = 4


def preprocess(x, edge_index, batch, W_emb, b_emb, conv_W, conv_b,
               W1, b1, W2, b2, W3, b3, n_cores=8, G=None, NCHK=5):
    x = np.asarray(x, np.float32)
    ei = np.asarray(edge_index, np.int64)
    batch = np.asarray(batch, np.int64)
    N, F = x.shape
    H = int(np.asarray(W_emb).shape[1])
    L = int(np.asarray(conv_W).shape[0])
    C = n_cores
    NS = N // C
    NB = (NS + 127) // 128
    NBF = (N + 127) // 128

    if G is None:
        G = int(batch.max()) + 1 if batch.size else 1

    loop = np.arange(N, dtype=np.int64)
    deg = (np.bincount(np.concatenate([ei[1], loop]), minlength=N)
           .astype(np.float64))
    dinv = (1.0 / np.sqrt(np.maximum(deg, 1.0))).astype(np.float32)
    src = np.concatenate([ei[0], loop])
    dst = np.concatenate([ei[1], loop])

    # ---- table ("AG") layout: chunk-major, core-major within chunk ----
    BPC = (NB + NCHK - 1) // NCHK
    CSZ = BPC * 128
    rk = np.array([min(NS, (k + 1) * CSZ) - min(NS, k * CSZ)
                   for k in range(NCHK)], np.int64)
    base = np.zeros(NCHK, np.int64)
    base[1:] = np.cumsum(C * rk)[:-1]

    def tablepos(v):
        c2 = v // NS
        r = v % NS
        k = np.minimum(r // CSZ, NCHK - 1)
        return base[k] + c2 * rk[k] + (r - k * CSZ)

    tp_all = tablepos(np.arange(N, dtype=np.int64))
    perm = np.zeros(N, np.int64)
    perm[tp_all] = np.arange(N)          # perm[tablepos] = original node

    # per-edge gather coordinates
    tp_src = tablepos(src)
    tok = tp_src // 2
    par = (tp_src % 2).astype(np.int64)
    half = tok // HTOK
    loc = tok % HTOK
    i16 = ((loc % NSTRH) * 128 + loc // NSTRH).astype(np.int16)
    norm = (dinv[src] * dinv[dst]).astype(np.float32)

    core = dst // NS
    block = (dst % NS) // 128
    dloc = (dst % NS - block * 128).astype(np.float32)

    # sort edges by (core, block, half, parity)
    order = np.lexsort((par, half, block, core))
    core_s = core[order]
    block_s = block[order]
    half_s = half[order]
    par_s = par[order]
    i16_s = i16[order]
    dloc_s = dloc[order]
    norm_s = norm[order]

    in_maps = []
    block_calls_all = None
    block_par_all = None
    T_tot_all = None
    idx_cols_all = None

    per_core = []
    for c in range(C):
        msk = core_s == c
        per_core.append((block_s[msk], half_s[msk], par_s[msk],
                         i16_s[msk], dloc_s[msk], norm_s[msk]))

    # build per-core tile lists; tbb/calls must be IDENTICAL across cores
    # (same program), so compute per-(block, half, parity) padded tile
    # counts as the max over cores.
    nsub = np.zeros((C, NB, 2, 2), np.int64)
    for c in range(C):
        bs, hs, ps, _, _, _ = per_core[c]
        np.add.at(nsub, (c, bs, hs, ps), 1)
    tsub = (nsub.max(axis=0) + 127) // 128          # [NB, 2, 2] tiles
    tsub = np.maximum(tsub, 0)

    block_calls = []
    block_par = []
    for b in range(NB):
        calls = []
        pars = []
        for h in range(2):
            tl = []                                  # parities per tile
            for p in range(2):
                tl += [p] * int(tsub[b, h, p])
            for o in range(0, len(tl), MAXCALL):
                nt = min(MAXCALL, len(tl) - o)
                calls.append((nt, h))
            pars += tl
        block_calls.append(tuple(calls))
        block_par.append(tuple(pars))
    T_tot = int(sum(len(p) for p in block_par))

    iota128 = np.tile(np.arange(128, dtype=np.float32), (128, 1)).astype(BF)
    iotag = np.tile(np.arange(G, dtype=np.float32), (128, 1)).astype(BF)
    ident = np.eye(128, dtype=np.float32).astype(BF)
    onesrow = np.ones((1, 128), F16)

    # replicated-embed inputs, permuted to table order, fp16
    xT1 = np.ones((F + 1, N), np.float32)
    xT1[:F] = x.T
    xT1 = np.ascontiguousarray(xT1[:, perm]).astype(F16)
    wemb1 = np.concatenate(
        [np.asarray(W_emb, np.float32),
         np.asarray(b_emb, np.float32).reshape(1, H)], axis=0).astype(F16)

    conv_W = np.asarray(conv_W, np.float32)
    conv_b = np.asarray(conv_b, np.float32)

    cntg = np.bincount(batch, minlength=G).astype(np.float32)
    invc = np.tile((1.0 / np.maximum(cntg, 1.0))[None, :],
                   (64, 1)).astype(np.float32)

    meta = Meta(N=N, F=F, H=H, G=G, L=L, C=C, NS=NS, NB=NB, NBF=NBF,
                T_tot=T_tot, NCHK=NCHK, BPC=BPC,
                block_calls=tuple(block_calls), block_par=tuple(block_par))

    for c in range(C):
        bs, hs, ps, i16c, dlc, nmc = per_core[c]
        # bucket edges
        buckets = {}
        for b in range(NB):
            for h in range(2):
                for p in range(2):
                    buckets[(b, h, p)] = []
        bsel = {}
        for key in buckets:
            b, h, p = key
            m = (bs == b) & (hs == h) & (ps == p)
            bsel[key] = (i16c[m], dlc[m], nmc[m])

        idxw_parts = []
        dstloc_arr = np.full((128, T_tot), -1.0, np.float32)
        norm_arr = np.zeros((128, T_tot), np.float32)
        t_glob = 0
        for b in range(NB):
            for h in range(2):
                # concatenated (parity 0 tiles, parity 1 tiles) edge stream
                tiles_idx = []
                for p in range(2):
                    ii, dd, nn = bsel[(b, h, p)]
                    ntile = int(tsub[b, h, p])
                    n = len(ii)
                    padn = ntile * 128
                    ip = np.zeros(padn, np.int16)
                    ip[:n] = ii
                    if n < padn:
                        ip[n:] = ii[-1] if n else 0
                    dp = np.full(padn, -1.0, np.float32)
                    dp[:n] = dd
                    np_ = np.zeros(padn, np.float32)
                    np_[:n] = nn
                    for t in range(ntile):
                        sl = slice(t * 128, (t + 1) * 128)
                        dstloc_arr[:, t_glob] = dp[sl]
                        norm_arr[:, t_glob] = np_[sl]
                        tiles_idx.append(ip[sl])
                        t_glob += 1
                # gather calls over these tiles (<=MAXCALL each)
                for o in range(0, len(tiles_idx), MAXCALL):
                    grp = tiles_idx[o:o + MAXCALL]
                    flat = np.concatenate(grp)
                    w = flat.reshape(len(flat) // 16, 16).T
                    idxw_parts.append(np.tile(w, (8, 1)))
        assert t_glob == T_tot
        idxw = np.concatenate(idxw_parts, axis=1)
        assert idxw.shape == (128, T_tot * 8)

        poolid = np.full((128, NB), -1.0, np.float32)
        for b in range(NB):
            w = min(128, NS - b * 128)
            poolid[:w, b] = batch[c * NS + b * 128: c * NS + b * 128 + w]

        m = {
            "xt1": xT1,
            "idxw": np.ascontiguousarray(idxw),
            "dstloc": dstloc_arr,
            "norme": norm_arr,
            "poolid": poolid,
            "iota128": iota128,
            "iotag": iotag,
            "ident": ident,
            "onesrow": onesrow,
            "wemb1": wemb1,
            "invc": invc,
            "w1": np.asarray(W1, np.float32),
            "b1": np.asarray(b1, np.float32).reshape(-1, 1),
            "w2": np.asarray(W2, np.float32),
            "b2": np.asarray(b2, np.float32).reshape(-1, 1),
            "w3": np.asarray(W3, np.float32),
            "b3": np.asarray(b3, np.float32).reshape(1, 1),
        }
        for i in range(L):
            m[f"cw_{i}"] = conv_W[i].astype(F16)
            m[f"cb_{i}"] = conv_b[i].reshape(1, H).astype(F16)
        in_maps.append(m)
    return meta, in_maps


def build_nc(meta: Meta, repeats=1):
    N, F, H, G, L, C = meta.N, meta.F, meta.H, meta.G, meta.L, meta.C
    NS, NB = meta.NS, meta.NB
    T_tot = meta.T_tot
    NCHK, BPC, nq = meta.NCHK, meta.BPC, meta.nq
    NPAD = 2 * 2 * HTOK                  # padded table rows (100352)
    CSZ = BPC * 128

    nc = bacc.Bacc("TRN2", target_bir_lowering=False, debug=False,
                   num_devices=C, num_swdge_queues=nq,
                   dynamic_dma_scratch_size=16384)

    def EIN(name, shape, dt):
        return nc.dram_tensor(name, list(shape), dt, kind="ExternalInput")

    xt1 = EIN("xt1", [F + 1, N], FP16)
    idxw = EIN("idxw", [128, T_tot * 8], I16)
    dstloc = EIN("dstloc", [128, T_tot], F32)
    norme = EIN("norme", [128, T_tot], F32)
    poolid = EIN("poolid", [128, NB], F32)
    iota128 = EIN("iota128", [128, 128], BF16)
    iotag = EIN("iotag", [128, G], BF16)
    ident = EIN("ident", [128, 128], BF16)
    onesrow = EIN("onesrow", [1, 128], FP16)
    wemb1 = EIN("wemb1", [F + 1, H], FP16)
    invc = EIN("invc", [64, G], F32)
    w1 = EIN("w1", [H, H], F32)
    b1 = EIN("b1", [H, 1], F32)
    w2 = EIN("w2", [H, H // 2], F32)
    b2 = EIN("b2", [H // 2, 1], F32)
    w3 = EIN("w3", [H // 2, 1], F32)
    b3 = EIN("b3", [1, 1], F32)
    cw = [EIN(f"cw_{i}", [H, H], FP16) for i in range(L)]
    cb = [EIN(f"cb_{i}", [1, H], FP16) for i in range(L)]

    out_d = nc.dram_tensor("out", [1, G], F32, kind="ExternalOutput")

    table0 = nc.dram_tensor("table0", [NPAD, H], BF16)
    table1 = nc.dram_tensor("table1", [NPAD, H], BF16, addr_space="Shared")
    table2 = nc.dram_tensor("table2", [NPAD, H], BF16, addr_space="Shared")
    bounce = nc.dram_tensor("bounce", [NS, H], BF16)
    pool_in = nc.dram_tensor("pool_in", [H, G], F32)
    pool_out = nc.dram_tensor("pool_out", [H, G], F32, addr_space="Shared")

    groups = [list(range(C))]
    chunk_lim = [(min(NS, k * CSZ), min(NS, (k + 1) * CSZ))
                 for k in range(NCHK)]

    gq = [0]

    with tile.TileContext(nc) as tc:
        import contextlib
        ctx = contextlib.ExitStack()
        with ctx:
            P = ctx.enter_context
            persist = P(tc.tile_pool(name="persist", bufs=1))
            gpool = P(tc.tile_pool(name="gpool", bufs=12))
            mtpool = P(tc.tile_pool(name="mtpool", bufs=6))
            ohpool = P(tc.tile_pool(name="ohpool", bufs=16))
            hap = P(tc.tile_pool(name="hap", bufs=3))
            stpool = P(tc.tile_pool(name="stpool", bufs=4))
            pohpool = P(tc.tile_pool(name="pohpool", bufs=3))
            bp_ps = P(tc.tile_pool(name="bp_ps", bufs=3, space="PSUM"))
            tr_ps = P(tc.tile_pool(name="tr_ps", bufs=2, space="PSUM"))
            st_ps = P(tc.tile_pool(name="st_ps", bufs=2, space="PSUM"))

            nc.gpsimd.load_library(library_config.mlp)

            def load(name, ap, shape, dt):
                t = persist.tile(list(shape), dt, tag=name)
                nc.sync.dma_start(out=t[:], in_=ap[:])
                return t

            idx_sb = load("idx_sb", idxw, [128, T_tot * 8], I16)
            dstloc_sb = load("dstloc_sb", dstloc, [128, T_tot], F32)
            norme_sb = load("norme_sb", norme, [128, T_tot], F32)
            poolid_sb = load("poolid_sb", poolid, [128, NB], F32)
            iota_sb = load("iota_sb", iota128, [128, 128], BF16)
            iotag_sb = load("iotag_sb", iotag, [128, G], BF16)
            ident_sb = load("ident_sb", ident, [128, 128], BF16)
            ones_sb = load("ones_sb", onesrow, [1, 128], FP16)
            wemb1_sb = load("wemb1_sb", wemb1, [F + 1, H], FP16)
            invc_sb = load("invc_sb", invc, [64, G], F32)
            w1_sb = load("w1_sb", w1, [H, H], F32)
            b1_sb = load("b1_sb", b1, [H, 1], F32)
            w2_sb = load("w2_sb", w2, [H, H // 2], F32)
            b2_sb = load("b2_sb", b2, [H // 2, 1], F32)
            w3_sb = load("w3_sb", w3, [H // 2, 1], F32)
            b3_sb = load("b3_sb", b3, [1, 1], F32)
            cw_sb = [load(f"cw_{i}_sb", cw[i], [H, H], FP16)
                     for i in range(L)]
            cb_sb = [load(f"cb_{i}_sb", cb[i], [1, H], FP16)
                     for i in range(L)]

            tab_sb = persist.tile([128, 2 * HTOK], BF16, tag="tab_sb")
            h3n = persist.tile([128, NB * H], BF16, tag="h3n")

            def gather(g_ap, half, c0, nidx):
                nc.gpsimd.dma_gather(
                    g_ap, tab_sb[:, half * HTOK:(half + 1) * HTOK],
                    idx_sb[:, c0:c0 + nidx // 16], nidx, nidx, 128,
                    transpose=True,
                    sbuf_tokens_per_rank=128,
                    sbuf_free_dim_per_rank=256,
                    sbuf_free_dim_pad_per_rank=0,
                    sbuf_byte_offset=0,
                    queue_num=gq[0] % nq)
                gq[0] += 1

            for _rep in range(repeats):
                # ======== embed (replicated, table order, fp16) ========
                XCH = 512
                with tc.tile_pool(name="xpool", bufs=3) as xpool:
                    for c0 in range(0, N, XCH):
                        cwd = min(XCH, N - c0)
                        xt = xpool.tile([F + 1, XCH], FP16, tag="xt")
                        nc.sync.dma_start(out=xt[:, :cwd],
                                          in_=xt1[:, c0:c0 + cwd])
                        for o in range(0, cwd, 128):
                            w = min(128, cwd - o)
                            ps = st_ps.tile([128, H], F32, tag="stps")
                            nc.tensor.matmul(out=ps[:w, :],
                                             lhsT=xt[:, o:o + w],
                                             rhs=wemb1_sb[:],
                                             start=True, stop=True)
                            st = stpool.tile([128, H], BF16, tag="st")
                            nc.scalar.activation(
                                out=st[:w, :], in_=ps[:w, :],
                                func=mybir.ActivationFunctionType.Relu)
                            nc.sync.dma_start(
                                out=table0[c0 + o:c0 + o + w, :],
                                in_=st[:w, :])

                # ======== conv layers ========
                tables = [table0, table1, table2]
                for li in range(L):
                    t_in = tables[li]
                    # load table into SBUF (blocked token layout, per half)
                    for h in range(2):
                        nc.sync.dma_start(
                            out=tab_sb[:, h * HTOK:(h + 1) * HTOK],
                            in_=t_in[h * 2 * HTOK:(h + 1) * 2 * HTOK, :]
                            .rearrange("(p s) e -> p (s e)", p=128))
                    t_glob = 0
                    c_glob = 0     # idx column (wrapped, /16)
                    for b in range(NB):
                        w = min(128, NS - b * 128)
                        calls = meta.block_calls[b]
                        pars = meta.block_par[b]
                        tb = len(pars)
                        ps_agg = bp_ps.tile([64, 128], F32, tag="bps")
                        # gather all tiles of this block
                        gts = []    # (gtile, j_in_call) per tile
                        for (nt, h) in calls:
                            g = gpool.tile([128, MAXCALL * 128], BF16,
                                           tag="g")
                            gather(g[:, :nt * 128]
                                   .rearrange("p (c e) -> p c e", c=1),
                                   h, c_glob, nt * 128)
                            c_glob += nt * 8
                            for j in range(nt):
                                gts.append((g, j))
                        # process tiles in groups of 4 (one psum bank):
                        # transpose each tile via out = g.T @ I (fp32 psum)
                        ti = 0
                        for g0 in range(0, tb, 4):
                            grp = gts[g0:g0 + 4]
                            ng = len(grp)
                            trp = tr_ps.tile([128, 4 * 128], F32,
                                             tag="trp")
                            for jj, (g, j) in enumerate(grp):
                                nc.tensor.matmul(
                                    out=trp[:, jj * 128:(jj + 1) * 128],
                                    lhsT=g[:, j * 128:(j + 1) * 128],
                                    rhs=ident_sb[:],
                                    start=True, stop=True)
                            mt = mtpool.tile([128, 4 * 128], BF16, tag="mt")
                            nc.scalar.activation(
                                out=mt[:, :ng * 128], in_=trp[:, :ng * 128],
                                func=mybir.ActivationFunctionType.Copy)
                            for jj in range(ng):
                                pi = pars[g0 + jj]
                                tt = t_glob + g0 + jj
                                oh = ohpool.tile([128, 128], BF16, tag="oh")
                                nc.vector.tensor_scalar(
                                    out=oh[:], in0=iota_sb[:],
                                    scalar1=dstloc_sb[:, tt:tt + 1],
                                    scalar2=norme_sb[:, tt:tt + 1],
                                    op0=mybir.AluOpType.is_equal,
                                    op1=mybir.AluOpType.mult)
                                nc.tensor.matmul(
                                    out=ps_agg[:],
                                    lhsT=mt[:, jj * 128 + pi * 64:
                                            jj * 128 + pi * 64 + 64],
                                    rhs=oh[:],
                                    start=(ti == 0), stop=(ti == tb - 1))
                                ti += 1
                        t_glob += tb
                        # block transform: h' = relu(agg @ W + b)
                        ha = hap.tile([64, 128], FP16, tag="ha")
                        nc.vector.tensor_copy(out=ha[:, :], in_=ps_agg[:])
                        ps2 = st_ps.tile([128, H], F32, tag="stps")
                        nc.tensor.matmul(out=ps2[:w, :], lhsT=ha[:, :w],
                                         rhs=cw_sb[li][:],
                                         start=True, stop=False)
                        nc.tensor.matmul(out=ps2[:w, :],
                                         lhsT=ones_sb[:, :w],
                                         rhs=cb_sb[li][:],
                                         start=False, stop=True)
                        if li < L - 1:
                            st = stpool.tile([128, H], BF16, tag="st")
                            nc.scalar.activation(
                                out=st[:w, :], in_=ps2[:w, :],
                                func=mybir.ActivationFunctionType.Relu)
                            nc.sync.dma_start(
                                out=bounce[b * 128:b * 128 + w, :],
                                in_=st[:w, :])
                            if b == NB - 1:
                                # all chunk AllGathers fired together at
                                # layer end: keeps the gather stream on
                                # gpsimd free of mid-layer sem stalls
                                for k in range(NCHK):
                                    r0, r1 = chunk_lim[k]
                                    if r1 > r0:
                                        t_out = tables[li + 1]
                                        nc.gpsimd.collective_compute(
                                            "AllGather",
                                            mybir.AluOpType.bypass,
                                            replica_groups=groups,
                                            ins=[bounce[r0:r1, :]],
                                            outs=[t_out[C * r0:C * r1, :]])
                        else:
                            nc.scalar.activation(
                                out=h3n[:w, b * H:(b + 1) * H],
                                in_=ps2[:w, :],
                                func=mybir.ActivationFunctionType.Relu)

                # ======== pooling ========
                with tc.tile_pool(name="pool_ps", bufs=1,
                                  space="PSUM") as pool_ps:
                    pps = pool_ps.tile([64, G], F32, tag="pps")
                    for b in range(NB):
                        w = min(128, NS - b * 128)
                        ohp = pohpool.tile([128, G], BF16, tag="ohp")
                        nc.vector.tensor_scalar(
                            out=ohp[:w, :], in0=iotag_sb[:w, :],
                            scalar1=poolid_sb[:w, b:b + 1], scalar2=None,
                            op0=mybir.AluOpType.is_equal)
                        nc.tensor.matmul(out=pps[:],
                                         lhsT=h3n[:w, b * H:(b + 1) * H],
                                         rhs=ohp[:w, :], start=(b == 0),
                                         stop=(b == NB - 1))
                    psum_sb = persist.tile([64, G], F32, tag="psum_sb")
                    nc.vector.tensor_copy(out=psum_sb[:], in_=pps[:])
                nc.sync.dma_start(out=pool_in[:], in_=psum_sb[:])
                nc.gpsimd.collective_compute(
                    "AllReduce", mybir.AluOpType.add, replica_groups=groups,
                    ins=[pool_in[:]], outs=[pool_out[:]])
                pooled = persist.tile([64, G], F32, tag="pooled")
                nc.sync.dma_start(out=pooled[:], in_=pool_out[:])
                nc.vector.tensor_tensor(out=pooled[:], in0=pooled[:],
                                        in1=invc_sb[:],
                                        op=mybir.AluOpType.mult)
                # ======== MLP ========
                with tc.tile_pool(name="mlp_ps", bufs=1,
                                  space="PSUM") as mlp_ps:
                    ps1 = mlp_ps.tile([64, G], F32, tag="mlpps")
                    nc.tensor.matmul(out=ps1[:, :G], lhsT=w1_sb[:],
                                     rhs=pooled[:], start=True, stop=True)
                    r1 = persist.tile([64, G], F32, tag="r1")
                    nc.scalar.activation(
                        out=r1[:], in_=ps1[:64, :G],
                        func=mybir.ActivationFunctionType.Relu,
                        bias=b1_sb[:, 0:1])
                    ps2m = mlp_ps.tile([64, G], F32, tag="mlpps")
                    nc.tensor.matmul(out=ps2m[:32, :G], lhsT=w2_sb[:],
                                     rhs=r1[:], start=True, stop=True)
                    r2 = persist.tile([32, G], F32, tag="r2")
                    nc.scalar.activation(
                        out=r2[:], in_=ps2m[:32, :G],
                        func=mybir.ActivationFunctionType.Relu,
                        bias=b2_sb[:, 0:1])
                    ps3 = mlp_ps.tile([64, G], F32, tag="mlpps")
                    nc.tensor.matmul(out=ps3[:1, :G], lhsT=w3_sb[:],
                                     rhs=r2[:], start=True, stop=True)
                    outs = persist.tile([1, G], F32, tag="outs")
                    nc.vector.tensor_scalar(out=outs[:], in0=ps3[:1, :G],
                                            scalar1=b3_sb[0:1, 0:1],
                                            scalar2=None,
                                            op0=mybir.AluOpType.add)
                nc.sync.dma_start(out=out_d[:], in_=outs[:])

    nc.compile()
    return nc


class SpmdRunner:
    def __init__(self, nc, n_cores):
        install_neuronx_cc_hook()
        self.nc = nc
        self.n_cores = n_cores
        partition_name = (nc.partition_id_tensor.name
                          if nc.partition_id_tensor else None)
        in_names, out_names, out_avals, zero_outs = [], [], [], []
        for alloc in nc.m.functions[0].allocations:
            if not isinstance(alloc, mybir.MemoryLocationSet):
                continue
            name = alloc.memorylocations[0].name
            if alloc.kind == "ExternalInput":
                if name != partition_name:
                    in_names.append(name)
            elif alloc.kind == "ExternalOutput":
                shape = tuple(alloc.tensor_shape)
                dt = mybir.dt.np(alloc.dtype)
                out_names.append(name)
                out_avals.append(jax.core.ShapedArray(shape, dt))
                zero_outs.append(np.zeros(shape, dt))
        self.in_names, self.out_names = in_names, out_names
        self.zero_outs = zero_outs
        bind_in_names = in_names + out_names
        if partition_name is not None:
            bind_in_names.append(partition_name)

        def _body(*args):
            operands = list(args)
            if partition_name is not None:
                operands.append(bass2jax.partition_id_tensor())
            outs = _bass_exec_p.bind(
                *operands,
                out_avals=tuple(out_avals),
                in_names=tuple(bind_in_names),
                out_names=tuple(out_names),
                lowering_input_output_aliases=(),
                sim_require_finite=False,
                sim_require_nnan=False,
                nc=nc,
            )
            return tuple(outs)

        devices = jax.devices()[:n_cores]
        self.mesh = Mesh(np.asarray(devices), ("core",))
        n_args = len(in_names) + len(zero_outs)
        in_specs = (PartitionSpec("core"),) * n_args
        out_specs = (PartitionSpec("core"),) * len(out_names)
        self.fn = jax.jit(
            shard_map(_body, mesh=self.mesh, in_specs=in_specs,
                      out_specs=out_specs, check_rep=False),
            keep_unused=True,
        )
        self._dev_in = None

    def set_inputs(self, in_maps):
        assert len(in_maps) == self.n_cores
        concat = [np.concatenate([np.asarray(in_maps[c][n])
                                  for c in range(self.n_cores)], axis=0)
                  for n in self.in_names]
        self._dev_in = [jax.device_put(a) for a in concat]
        self._dev_zeros = [
            jax.device_put(np.zeros((self.n_cores * z.shape[0], *z.shape[1:]),
                                    z.dtype)) for z in self.zero_outs]
        jax.block_until_ready(self._dev_in)

    def run(self):
        outs = self.fn(*self._dev_in, *self._dev_zeros)
        jax.block_until_ready(outs)
        return outs

    def results(self, outs):
        res = [dict() for _ in range(self.n_cores)]
        for i, name in enumerate(self.out_names):
            arr = np.asarray(outs[i])
            per = np.split(arr, self.n_cores, axis=0)
            for c in range(self.n_cores):
                res[c][name] = per[c]
        return res


_CACHE = {}


def _get_runner(meta, in_maps, repeats=1):
    key = (meta.N, meta.T_tot, meta.NCHK, hash(meta.block_calls),
           hash(meta.block_par), repeats)
    if key not in _CACHE:
        nc = build_nc(meta, repeats=repeats)
        _CACHE[key] = SpmdRunner(nc, meta.C)
    return _CACHE[key]


def kernel(x, edge_index, batch, W_emb, b_emb, conv_W, conv_b,
           W1, b1, W2, b2, W3, b3):
    """Full (unsharded) inputs -> full [G, 1] float32 output."""
    G = 256
    meta, in_maps = preprocess(
        x, edge_index, batch, W_emb, b_emb, conv_W, conv_b,
        W1, b1, W2, b2, W3, b3, n_cores=8, G=G)
    r = _get_runner(meta, in_maps)
    r.set_inputs(in_maps)
    res = r.results(r.run())
    return np.ascontiguousarray(res[0]["out"].reshape(G, 1).astype(np.float32))


# revision 12
# speedup vs baseline: 2.6266x; 1.2515x over previous
"""Self-contained TRN2 Bass kernel for the GCN message-passing problem.

8-core SPMD, v6 (SBUF-table gather architecture):
- Nodes sharded by dst across cores (NS = N/C per core). GCN norm
  (dinv_src*dinv_dst) folded into the one-hot selection matrices.
- Per-layer node-feature table h (bf16, [Npad, 64] DRAM, core/chunk-major
  "AG layout") viewed as 50176 tokens of 128 bf16 (2 nodes per token).
  Each layer the full table is loaded into SBUF (98KB/partition, blocked:
  token q -> partition loc//196, stripe loc%196, per 25088-token half).
- Edges grouped by (dst block, half, parity) into 128-edge tiles; tiles
  gathered from the SBUF table with custom transpose-mode dma_gather
  (<=512 rows/call, int16 indices, round-robin over 4 SWDGE queues:
  ~0.9ns/row). Gathered columns are feature-major; per tile: PE transpose
  (bf16, into grouped psum), ACT copy psum->SBUF, DVE one-hot
  (is_equal x norm), PE matmul psum[64f,128d] += mt[128e,64].T @ oh.
- Per-block transform: psum_agg -> fp16 tile; h' = relu(agg @ W + b) via
  two fp16 matmuls (ones-row trick for bias) -> node-major psum -> ACT
  relu -> bf16 -> bounce. Tables for layers 1,2 distributed via chunked
  AllGathers interleaved with block processing. Layer-0 table computed
  fully on every core (replicated embed, fp16, no collective).
- Pooling: one-hot over G graphs into psum [64, G], AllReduce, small fp32
  MLP on every core; core 0's output used.
"""
from dataclasses import dataclass

import numpy as np
import jax
import ml_dtypes
from jax.sharding import Mesh, PartitionSpec
from jax.experimental.shard_map import shard_map

from concourse import bass2jax
from concourse.bass2jax import _bass_exec_p, install_neuronx_cc_hook
from concourse import library_config

import concourse.bass as bass
import concourse.bacc as bacc
import concourse.mybir as mybir
import concourse.tile as tile

F32 = mybir.dt.float32
BF16 = mybir.dt.bfloat16
FP16 = mybir.dt.float16
I16 = mybir.dt.int16

BF = ml_dtypes.bfloat16
F16 = np.float16

HTOK = 25088          # tokens per half (196 stripes x 128 partitions)
NSTRH = 196           # stripes per half per partition
MAXCALL = 4           # tiles per dma_gather call (<=512 idx)


@dataclass
class Meta:
    N: int
    F: int
    H: int
    G: int
    L: int
    C: int
    NS: int
    NB: int
    NBF: int
    T_tot: int
    NCHK: int
    BPC: int
    block_calls: tuple   # per block: tuple of (nt, half)
    block_par: tuple     # per block: tuple of parity per tile
    nq: int = 4


def preprocess(x, edge_index, batch, W_emb, b_emb, conv_W, conv_b,
               W1, b1, W2, b2, W3, b3, n_cores=8, G=None, NCHK=1):
    x = np.asarray(x, np.float32)
    ei = np.asarray(edge_index, np.int64)
    batch = np.asarray(batch, np.int64)
    N, F = x.shape
    H = int(np.asarray(W_emb).shape[1])
    L = int(np.asarray(conv_W).shape[0])
    C = n_cores
    NS = N // C
    NB = (NS + 127) // 128
    NBF = (N + 127) // 128

    if G is None:
        G = int(batch.max()) + 1 if batch.size else 1

    loop = np.arange(N, dtype=np.int64)
    deg = (np.bincount(np.concatenate([ei[1], loop]), minlength=N)
           .astype(np.float64))
    dinv = (1.0 / np.sqrt(np.maximum(deg, 1.0))).astype(np.float32)
    src = np.concatenate([ei[0], loop])
    dst = np.concatenate([ei[1], loop])

    # ---- table ("AG") layout: chunk-major, core-major within chunk ----
    BPC = (NB + NCHK - 1) // NCHK
    CSZ = BPC * 128
    rk = np.array([min(NS, (k + 1) * CSZ) - min(NS, k * CSZ)
                   for k in range(NCHK)], np.int64)
    base = np.zeros(NCHK, np.int64)
    base[1:] = np.cumsum(C * rk)[:-1]

    def tablepos(v):
        c2 = v // NS
        r = v % NS
        k = np.minimum(r // CSZ, NCHK - 1)
        return base[k] + c2 * rk[k] + (r - k * CSZ)

    tp_all = tablepos(np.arange(N, dtype=np.int64))
    perm = np.zeros(N, np.int64)
    perm[tp_all] = np.arange(N)          # perm[tablepos] = original node

    # per-edge gather coordinates
    tp_src = tablepos(src)
    tok = tp_src // 2
    par = (tp_src % 2).astype(np.int64)
    half = tok // HTOK
    loc = tok % HTOK
    i16 = ((loc % NSTRH) * 128 + loc // NSTRH).astype(np.int16)
    norm = (dinv[src] * dinv[dst]).astype(np.float32)

    core = dst // NS
    block = (dst % NS) // 128
    dloc = (dst % NS - block * 128).astype(np.float32)

    # sort edges by (core, block, half, parity)
    order = np.lexsort((par, half, block, core))
    core_s = core[order]
    block_s = block[order]
    half_s = half[order]
    par_s = par[order]
    i16_s = i16[order]
    dloc_s = dloc[order]
    norm_s = norm[order]

    in_maps = []
    block_calls_all = None
    block_par_all = None
    T_tot_all = None
    idx_cols_all = None

    per_core = []
    for c in range(C):
        msk = core_s == c
        per_core.append((block_s[msk], half_s[msk], par_s[msk],
                         i16_s[msk], dloc_s[msk], norm_s[msk]))

    # build per-core tile lists; tbb/calls must be IDENTICAL across cores
    # (same program), so compute per-(block, half, parity) padded tile
    # counts as the max over cores.
    nsub = np.zeros((C, NB, 2, 2), np.int64)
    for c in range(C):
        bs, hs, ps, _, _, _ = per_core[c]
        np.add.at(nsub, (c, bs, hs, ps), 1)
    tsub = (nsub.max(axis=0) + 127) // 128          # [NB, 2, 2] tiles
    tsub = np.maximum(tsub, 0)

    block_calls = []
    block_par = []
    for b in range(NB):
        calls = []
        pars = []
        for h in range(2):
            tl = []                                  # parities per tile
            for p in range(2):
                tl += [p] * int(tsub[b, h, p])
            for o in range(0, len(tl), MAXCALL):
                nt = min(MAXCALL, len(tl) - o)
                calls.append((nt, h))
            pars += tl
        block_calls.append(tuple(calls))
        block_par.append(tuple(pars))
    T_tot = int(sum(len(p) for p in block_par))

    iota128 = np.tile(np.arange(128, dtype=np.float32), (128, 1)).astype(BF)
    iotag = np.tile(np.arange(G, dtype=np.float32), (128, 1)).astype(BF)
    ident = np.eye(128, dtype=np.float32).astype(BF)
    onesrow = np.ones((1, 128), F16)

    # replicated-embed inputs, permuted to table order, fp16
    xT1 = np.ones((F + 1, N), np.float32)
    xT1[:F] = x.T
    xT1 = np.ascontiguousarray(xT1[:, perm]).astype(F16)
    wemb1 = np.concatenate(
        [np.asarray(W_emb, np.float32),
         np.asarray(b_emb, np.float32).reshape(1, H)], axis=0).astype(F16)

    conv_W = np.asarray(conv_W, np.float32)
    conv_b = np.asarray(conv_b, np.float32)

    cntg = np.bincount(batch, minlength=G).astype(np.float32)
    invc = np.tile((1.0 / np.maximum(cntg, 1.0))[None, :],
                   (64, 1)).astype(np.float32)

    meta = Meta(N=N, F=F, H=H, G=G, L=L, C=C, NS=NS, NB=NB, NBF=NBF,
                T_tot=T_tot, NCHK=NCHK, BPC=BPC,
                block_calls=tuple(block_calls), block_par=tuple(block_par))

    for c in range(C):
        bs, hs, ps, i16c, dlc, nmc = per_core[c]
        # bucket edges
        buckets = {}
        for b in range(NB):
            for h in range(2):
                for p in range(2):
                    buckets[(b, h, p)] = []
        bsel = {}
        for key in buckets:
            b, h, p = key
            m = (bs == b) & (hs == h) & (ps == p)
            bsel[key] = (i16c[m], dlc[m], nmc[m])

        idxw_parts = []
        dstloc_arr = np.full((128, T_tot), -1.0, np.float32)
        norm_arr = np.zeros((128, T_tot), np.float32)
        t_glob = 0
        for b in range(NB):
            for h in range(2):
                # concatenated (parity 0 tiles, parity 1 tiles) edge stream
                tiles_idx = []
                for p in range(2):
                    ii, dd, nn = bsel[(b, h, p)]
                    ntile = int(tsub[b, h, p])
                    n = len(ii)
                    padn = ntile * 128
                    ip = np.zeros(padn, np.int16)
                    ip[:n] = ii
                    if n < padn:
                        ip[n:] = ii[-1] if n else 0
                    dp = np.full(padn, -1.0, np.float32)
                    dp[:n] = dd
                    np_ = np.zeros(padn, np.float32)
                    np_[:n] = nn
                    for t in range(ntile):
                        sl = slice(t * 128, (t + 1) * 128)
                        dstloc_arr[:, t_glob] = dp[sl]
                        norm_arr[:, t_glob] = np_[sl]
                        tiles_idx.append(ip[sl])
                        t_glob += 1
                # gather calls over these tiles (<=MAXCALL each)
                for o in range(0, len(tiles_idx), MAXCALL):
                    grp = tiles_idx[o:o + MAXCALL]
                    flat = np.concatenate(grp)
                    w = flat.reshape(len(flat) // 16, 16).T
                    idxw_parts.append(np.tile(w, (8, 1)))
        assert t_glob == T_tot
        idxw = np.concatenate(idxw_parts, axis=1)
        assert idxw.shape == (128, T_tot * 8)

        poolid = np.full((128, NB), -1.0, np.float32)
        for b in range(NB):
            w = min(128, NS - b * 128)
            poolid[:w, b] = batch[c * NS + b * 128: c * NS + b * 128 + w]

        m = {
            "xt1": xT1,
            "idxw": np.ascontiguousarray(idxw),
            "dstloc": dstloc_arr,
            "norme": norm_arr,
            "poolid": poolid,
            "iota128": iota128,
            "iotag": iotag,
            "ident": ident,
            "onesrow": onesrow,
            "wemb1": wemb1,
            "invc": invc,
            "w1": np.asarray(W1, np.float32),
            "b1": np.asarray(b1, np.float32).reshape(-1, 1),
            "w2": np.asarray(W2, np.float32),
            "b2": np.asarray(b2, np.float32).reshape(-1, 1),
            "w3": np.asarray(W3, np.float32),
            "b3": np.asarray(b3, np.float32).reshape(1, 1),
        }
        for i in range(L):
            m[f"cw_{i}"] = conv_W[i].astype(F16)
            m[f"cb_{i}"] = conv_b[i].reshape(1, H).astype(F16)
        in_maps.append(m)
    return meta, in_maps


def build_nc(meta: Meta, repeats=1):
    N, F, H, G, L, C = meta.N, meta.F, meta.H, meta.G, meta.L, meta.C
    NS, NB = meta.NS, meta.NB
    T_tot = meta.T_tot
    NCHK, BPC, nq = meta.NCHK, meta.BPC, meta.nq
    NPAD = 2 * 2 * HTOK                  # padded table rows (100352)
    CSZ = BPC * 128

    nc = bacc.Bacc("TRN2", target_bir_lowering=False, debug=False,
                   num_devices=C, num_swdge_queues=nq,
                   dynamic_dma_scratch_size=16384)

    def EIN(name, shape, dt):
        return nc.dram_tensor(name, list(shape), dt, kind="ExternalInput")

    xt1 = EIN("xt1", [F + 1, N], FP16)
    idxw = EIN("idxw", [128, T_tot * 8], I16)
    dstloc = EIN("dstloc", [128, T_tot], F32)
    norme = EIN("norme", [128, T_tot], F32)
    poolid = EIN("poolid", [128, NB], F32)
    iota128 = EIN("iota128", [128, 128], BF16)
    iotag = EIN("iotag", [128, G], BF16)
    ident = EIN("ident", [128, 128], BF16)
    onesrow = EIN("onesrow", [1, 128], FP16)
    wemb1 = EIN("wemb1", [F + 1, H], FP16)
    invc = EIN("invc", [64, G], F32)
    w1 = EIN("w1", [H, H], F32)
    b1 = EIN("b1", [H, 1], F32)
    w2 = EIN("w2", [H, H // 2], F32)
    b2 = EIN("b2", [H // 2, 1], F32)
    w3 = EIN("w3", [H // 2, 1], F32)
    b3 = EIN("b3", [1, 1], F32)
    cw = [EIN(f"cw_{i}", [H, H], FP16) for i in range(L)]
    cb = [EIN(f"cb_{i}", [1, H], FP16) for i in range(L)]

    out_d = nc.dram_tensor("out", [1, G], F32, kind="ExternalOutput")

    table0 = nc.dram_tensor("table0", [NPAD, H], BF16)
    table1 = nc.dram_tensor("table1", [NPAD, H], BF16, addr_space="Shared")
    table2 = nc.dram_tensor("table2", [NPAD, H], BF16, addr_space="Shared")
    bounce = nc.dram_tensor("bounce", [NS, H], BF16)
    pool_in = nc.dram_tensor("pool_in", [H, G], F32)
    pool_out = nc.dram_tensor("pool_out", [H, G], F32, addr_space="Shared")

    groups = [list(range(C))]
    chunk_lim = [(min(NS, k * CSZ), min(NS, (k + 1) * CSZ))
                 for k in range(NCHK)]

    gq = [0]

    with tile.TileContext(nc) as tc:
        import contextlib
        ctx = contextlib.ExitStack()
        with ctx:
            P = ctx.enter_context
            persist = P(tc.tile_pool(name="persist", bufs=1))
            gpool = P(tc.tile_pool(name="gpool", bufs=12))
            mtpool = P(tc.tile_pool(name="mtpool", bufs=6))
            ohpool = P(tc.tile_pool(name="ohpool", bufs=16))
            hap = P(tc.tile_pool(name="hap", bufs=3))
            stpool = P(tc.tile_pool(name="stpool", bufs=4))
            pohpool = P(tc.tile_pool(name="pohpool", bufs=3))
            bp_ps = P(tc.tile_pool(name="bp_ps", bufs=3, space="PSUM"))
            tr_ps = P(tc.tile_pool(name="tr_ps", bufs=2, space="PSUM"))
            st_ps = P(tc.tile_pool(name="st_ps", bufs=2, space="PSUM"))

            nc.gpsimd.load_library(library_config.mlp)

            def load(name, ap, shape, dt):
                t = persist.tile(list(shape), dt, tag=name)
                nc.sync.dma_start(out=t[:], in_=ap[:])
                return t

            idx_sb = load("idx_sb", idxw, [128, T_tot * 8], I16)
            dstloc_sb = load("dstloc_sb", dstloc, [128, T_tot], F32)
            norme_sb = load("norme_sb", norme, [128, T_tot], F32)
            poolid_sb = load("poolid_sb", poolid, [128, NB], F32)
            iota_sb = load("iota_sb", iota128, [128, 128], BF16)
            iotag_sb = load("iotag_sb", iotag, [128, G], BF16)
            ident_sb = load("ident_sb", ident, [128, 128], BF16)
            ones_sb = load("ones_sb", onesrow, [1, 128], FP16)
            wemb1_sb = load("wemb1_sb", wemb1, [F + 1, H], FP16)
            invc_sb = load("invc_sb", invc, [64, G], F32)
            w1_sb = load("w1_sb", w1, [H, H], F32)
            b1_sb = load("b1_sb", b1, [H, 1], F32)
            w2_sb = load("w2_sb", w2, [H, H // 2], F32)
            b2_sb = load("b2_sb", b2, [H // 2, 1], F32)
            w3_sb = load("w3_sb", w3, [H // 2, 1], F32)
            b3_sb = load("b3_sb", b3, [1, 1], F32)
            cw_sb = [load(f"cw_{i}_sb", cw[i], [H, H], FP16)
                     for i in range(L)]
            cb_sb = [load(f"cb_{i}_sb", cb[i], [1, H], FP16)
                     for i in range(L)]

            tab_sb = persist.tile([128, 2 * HTOK], BF16, tag="tab_sb")
            h3n = persist.tile([128, NB * H], BF16, tag="h3n")

            def gather(g_ap, half, c0, nidx):
                nc.gpsimd.dma_gather(
                    g_ap, tab_sb[:, half * HTOK:(half + 1) * HTOK],
                    idx_sb[:, c0:c0 + nidx // 16], nidx, nidx, 128,
                    transpose=True,
                    sbuf_tokens_per_rank=128,
                    sbuf_free_dim_per_rank=256,
                    sbuf_free_dim_pad_per_rank=0,
                    sbuf_byte_offset=0,
                    queue_num=gq[0] % nq)
                gq[0] += 1

            for _rep in range(repeats):
                # ======== embed (replicated, table order, fp16) ========
                XCH = 512
                with tc.tile_pool(name="xpool", bufs=3) as xpool:
                    for c0 in range(0, N, XCH):
                        cwd = min(XCH, N - c0)
                        xt = xpool.tile([F + 1, XCH], FP16, tag="xt")
                        nc.sync.dma_start(out=xt[:, :cwd],
                                          in_=xt1[:, c0:c0 + cwd])
                        for o in range(0, cwd, 128):
                            w = min(128, cwd - o)
                            ps = st_ps.tile([128, H], F32, tag="stps")
                            nc.tensor.matmul(out=ps[:w, :],
                                             lhsT=xt[:, o:o + w],
                                             rhs=wemb1_sb[:],
                                             start=True, stop=True)
                            st = stpool.tile([128, H], BF16, tag="st")
                            nc.scalar.activation(
                                out=st[:w, :], in_=ps[:w, :],
                                func=mybir.ActivationFunctionType.Relu)
                            nc.sync.dma_start(
                                out=table0[c0 + o:c0 + o + w, :],
                                in_=st[:w, :])

                # ======== conv layers ========
                tables = [table0, table1, table2]
                for li in range(L):
                    t_in = tables[li]
                    # load table into SBUF (blocked token layout, per half)
                    for h in range(2):
                        nc.sync.dma_start(
                            out=tab_sb[:, h * HTOK:(h + 1) * HTOK],
                            in_=t_in[h * 2 * HTOK:(h + 1) * 2 * HTOK, :]
                            .rearrange("(p s) e -> p (s e)", p=128))
                    t_glob = 0
                    c_glob = 0     # idx column (wrapped, /16)
                    for b in range(NB):
                        w = min(128, NS - b * 128)
                        calls = meta.block_calls[b]
                        pars = meta.block_par[b]
                        tb = len(pars)
                        ps_agg = bp_ps.tile([64, 128], F32, tag="bps")
                        # gather all tiles of this block
                        gts = []    # (gtile, j_in_call) per tile
                        for (nt, h) in calls:
                            g = gpool.tile([128, MAXCALL * 128], BF16,
                                           tag="g")
                            gather(g[:, :nt * 128]
                                   .rearrange("p (c e) -> p c e", c=1),
                                   h, c_glob, nt * 128)
                            c_glob += nt * 8
                            for j in range(nt):
                                gts.append((g, j))
                        # process tiles in groups of 4 (one psum bank):
                        # transpose each tile via out = g.T @ I (fp32 psum)
                        ti = 0
                        for g0 in range(0, tb, 4):
                            grp = gts[g0:g0 + 4]
                            ng = len(grp)
                            trp = tr_ps.tile([128, 4 * 128], F32,
                                             tag="trp")
                            for jj, (g, j) in enumerate(grp):
                                nc.tensor.matmul(
                                    out=trp[:, jj * 128:(jj + 1) * 128],
                                    lhsT=g[:, j * 128:(j + 1) * 128],
                                    rhs=ident_sb[:],
                                    start=True, stop=True)
                            mt = mtpool.tile([128, 4 * 128], BF16, tag="mt")
                            nc.scalar.activation(
                                out=mt[:, :ng * 128], in_=trp[:, :ng * 128],
                                func=mybir.ActivationFunctionType.Copy)
                            for jj in range(ng):
                                pi = pars[g0 + jj]
                                tt = t_glob + g0 + jj
                                oh = ohpool.tile([128, 128], BF16, tag="oh")
                                nc.vector.tensor_scalar(
                                    out=oh[:], in0=iota_sb[:],
                                    scalar1=dstloc_sb[:, tt:tt + 1],
                                    scalar2=norme_sb[:, tt:tt + 1],
                                    op0=mybir.AluOpType.is_equal,
                                    op1=mybir.AluOpType.mult)
                                nc.tensor.matmul(
                                    out=ps_agg[:],
                                    lhsT=mt[:, jj * 128 + pi * 64:
                                            jj * 128 + pi * 64 + 64],
                                    rhs=oh[:],
                                    start=(ti == 0), stop=(ti == tb - 1))
                                ti += 1
                        t_glob += tb
                        # block transform: h' = relu(agg @ W + b)
                        ha = hap.tile([64, 128], FP16, tag="ha")
                        nc.vector.tensor_copy(out=ha[:, :], in_=ps_agg[:])
                        ps2 = st_ps.tile([128, H], F32, tag="stps")
                        nc.tensor.matmul(out=ps2[:w, :], lhsT=ha[:, :w],
                                         rhs=cw_sb[li][:],
                                         start=True, stop=False)
                        nc.tensor.matmul(out=ps2[:w, :],
                                         lhsT=ones_sb[:, :w],
                                         rhs=cb_sb[li][:],
                                         start=False, stop=True)
                        if li < L - 1:
                            st = stpool.tile([128, H], BF16, tag="st")
                            nc.scalar.activation(
                                out=st[:w, :], in_=ps2[:w, :],
                                func=mybir.ActivationFunctionType.Relu)
                            nc.sync.dma_start(
                                out=bounce[b * 128:b * 128 + w, :],
                                in_=st[:w, :])
                            if b == NB - 1:
                                # all chunk AllGathers fired together at
                                # layer end: keeps the gather stream on
                                # gpsimd free of mid-layer sem stalls
                                for k in range(NCHK):
                                    r0, r1 = chunk_lim[k]
                                    if r1 > r0:
                                        t_out = tables[li + 1]
                                        nc.gpsimd.collective_compute(
                                            "AllGather",
                                            mybir.AluOpType.bypass,
                                            replica_groups=groups,
                                            ins=[bounce[r0:r1, :]],
                                            outs=[t_out[C * r0:C * r1, :]])
                        else:
                            nc.scalar.activation(
                                out=h3n[:w, b * H:(b + 1) * H],
                                in_=ps2[:w, :],
                                func=mybir.ActivationFunctionType.Relu)

                # ======== pooling ========
                with tc.tile_pool(name="pool_ps", bufs=1,
                                  space="PSUM") as pool_ps:
                    pps = pool_ps.tile([64, G], F32, tag="pps")
                    for b in range(NB):
                        w = min(128, NS - b * 128)
                        ohp = pohpool.tile([128, G], BF16, tag="ohp")
                        nc.vector.tensor_scalar(
                            out=ohp[:w, :], in0=iotag_sb[:w, :],
                            scalar1=poolid_sb[:w, b:b + 1], scalar2=None,
                            op0=mybir.AluOpType.is_equal)
                        nc.tensor.matmul(out=pps[:],
                                         lhsT=h3n[:w, b * H:(b + 1) * H],
                                         rhs=ohp[:w, :], start=(b == 0),
                                         stop=(b == NB - 1))
                    psum_sb = persist.tile([64, G], F32, tag="psum_sb")
                    nc.vector.tensor_copy(out=psum_sb[:], in_=pps[:])
                nc.sync.dma_start(out=pool_in[:], in_=psum_sb[:])
                nc.gpsimd.collective_compute(
                    "AllReduce", mybir.AluOpType.add, replica_groups=groups,
                    ins=[pool_in[:]], outs=[pool_out[:]])
                pooled = persist.tile([64, G], F32, tag="pooled")
                nc.sync.dma_start(out=pooled[:], in_=pool_out[:])
                nc.vector.tensor_tensor(out=pooled[:], in0=pooled[:],
                                        in1=invc_sb[:],
                                        op=mybir.AluOpType.mult)
                # ======== MLP ========
                with tc.tile_pool(name="mlp_ps", bufs=1,
                                  space="PSUM") as mlp_ps:
                    ps1 = mlp_ps.tile([64, G], F32, tag="mlpps")
                    nc.tensor.matmul(out=ps1[:, :G], lhsT=w1_sb[:],
                                     rhs=pooled[:], start=True, stop=True)
                    r1 = persist.tile([64, G], F32, tag="r1")
                    nc.scalar.activation(
                        out=r1[:], in_=ps1[:64, :G],
                        func=mybir.ActivationFunctionType.Relu,
                        bias=b1_sb[:, 0:1])
                    ps2m = mlp_ps.tile([64, G], F32, tag="mlpps")
                    nc.tensor.matmul(out=ps2m[:32, :G], lhsT=w2_sb[:],
                                     rhs=r1[:], start=True, stop=True)
                    r2 = persist.tile([32, G], F32, tag="r2")
                    nc.scalar.activation(
                        out=r2[:], in_=ps2m[:32, :G],
                        func=mybir.ActivationFunctionType.Relu,
                        bias=b2_sb[:, 0:1])
                    ps3 = mlp_ps.tile([64, G], F32, tag="mlpps")
                    nc.tensor.matmul(out=ps3[:1, :G], lhsT=w3_sb[:],
                                     rhs=r2[:], start=True, stop=True)
                    outs = persist.tile([1, G], F32, tag="outs")
                    nc.vector.tensor_scalar(out=outs[:], in0=ps3[:1, :G],
                                            scalar1=b3_sb[0:1, 0:1],
                                            scalar2=None,
                                            op0=mybir.AluOpType.add)
                nc.sync.dma_start(out=out_d[:], in_=outs[:])

    nc.compile()
    return nc


class SpmdRunner:
    def __init__(self, nc, n_cores):
        install_neuronx_cc_hook()
        self.nc = nc
        self.n_cores = n_cores
        partition_name = (nc.partition_id_tensor.name
                          if nc.partition_id_tensor else None)
        in_names, out_names, out_avals, zero_outs = [], [], [], []
        for alloc in nc.m.functions[0].allocations:
            if not isinstance(alloc, mybir.MemoryLocationSet):
                continue
            name = alloc.memorylocations[0].name
            if alloc.kind == "ExternalInput":
                if name != partition_name:
                    in_names.append(name)
            elif alloc.kind == "ExternalOutput":
                shape = tuple(alloc.tensor_shape)
                dt = mybir.dt.np(alloc.dtype)
                out_names.append(name)
                out_avals.append(jax.core.ShapedArray(shape, dt))
                zero_outs.append(np.zeros(shape, dt))
        self.in_names, self.out_names = in_names, out_names
        self.zero_outs = zero_outs
        bind_in_names = in_names + out_names
        if partition_name is not None:
            bind_in_names.append(partition_name)

        def _body(*args):
            operands = list(args)
            if partition_name is not None:
                operands.append(bass2jax.partition_id_tensor())
            outs = _bass_exec_p.bind(
                *operands,
                out_avals=tuple(out_avals),
                in_names=tuple(bind_in_names),
                out_names=tuple(out_names),
                lowering_input_output_aliases=(),
                sim_require_finite=False,
                sim_require_nnan=False,
                nc=nc,
            )
            return tuple(outs)

        devices = jax.devices()[:n_cores]
        self.mesh = Mesh(np.asarray(devices), ("core",))
        n_args = len(in_names) + len(zero_outs)
        in_specs = (PartitionSpec("core"),) * n_args
        out_specs = (PartitionSpec("core"),) * len(out_names)
        self.fn = jax.jit(
            shard_map(_body, mesh=self.mesh, in_specs=in_specs,
                      out_specs=out_specs, check_rep=False),
            keep_unused=True,
        )
        self._dev_in = None

    def set_inputs(self, in_maps):
        assert len(in_maps) == self.n_cores
        concat = [np.concatenate([np.asarray(in_maps[c][n])
                                  for c in range(self.n_cores)], axis=0)
                  for n in self.in_names]
        self._dev_in = [jax.device_put(a) for a in concat]
        self._dev_zeros = [
            jax.device_put(np.zeros((self.n_cores * z.shape[0], *z.shape[1:]),
                                    z.dtype)) for z in self.zero_outs]
        jax.block_until_ready(self._dev_in)

    def run(self):
        outs = self.fn(*self._dev_in, *self._dev_zeros)
        jax.block_until_ready(outs)
        return outs

    def results(self, outs):
        res = [dict() for _ in range(self.n_cores)]
        for i, name in enumerate(self.out_names):
            arr = np.asarray(outs[i])
            per = np.split(arr, self.n_cores, axis=0)
            for c in range(self.n_cores):
                res[c][name] = per[c]
        return res


_CACHE = {}


def _get_runner(meta, in_maps, repeats=1):
    key = (meta.N, meta.T_tot, meta.NCHK, hash(meta.block_calls),
           hash(meta.block_par), repeats)
    if key not in _CACHE:
        nc = build_nc(meta, repeats=repeats)
        _CACHE[key] = SpmdRunner(nc, meta.C)
    return _CACHE[key]


def kernel(x, edge_index, batch, W_emb, b_emb, conv_W, conv_b,
           W1, b1, W2, b2, W3, b3):
    """Full (unsharded) inputs -> full [G, 1] float32 output."""
    G = 256
    meta, in_maps = preprocess(
        x, edge_index, batch, W_emb, b_emb, conv_W, conv_b,
        W1, b1, W2, b2, W3, b3, n_cores=8, G=G)
    r = _get_runner(meta, in_maps)
    r.set_inputs(in_maps)
    res = r.results(r.run())
    return np.ascontiguousarray(res[0]["out"].reshape(G, 1).astype(np.float32))
